# revision 9
# baseline (speedup 1.0000x reference)
"""Head-parallel multi-head attention kernel for 8 TRN2 NeuronCores.

Problem: X[4096,1024] @ per-head Wq/Wk/Wv[1024,128] (+bias) -> per-head
scores S = q k^T * SCALE, softmax over the QUERY axis (axis n), z = attn @ v,
concat heads, sigmoid.  H=8 heads -> 1 head per core, zero collectives.

Per-core algorithm (head h):
  - Work in transposed space: T = S^T laid out [m(part), n(free)] so the
    softmax reduction is a free-axis row-sum.
  - softmax normalization folded into v:  z^T[e,n] = sum_m exp(T[m,n]) *
    (v[m,e]/denom[m]); denom comes free from the ACT accum_out of the exp.
  - zT accumulated in PSUM per 8-chunk m-block, flushed to SBUF by DVE.
  - sigmoid on zT, PE-transpose back to [n,e], DMA out.
Host: shard weights per head, replicate X, concat core outputs on axis=1.
"""

import numpy as np
import ml_dtypes

from concourse import bacc, bass, tile, mybir
from concourse.bass_utils import run_bass_kernel_spmd

N, D, E = 4096, 1024, 128
H = 8
SCALE = 0.08838834764831845
BF16 = mybir.dt.bfloat16
F32 = mybir.dt.float32

DC = D // 128      # 8 d-chunks of 128 (contraction tiles)
NG = N // 512      # 8 column groups of 512
MC = N // 128      # 32 m-chunks of 128
MB = MC // 8       # 4 m-blocks of 8 chunks
CH = N // 1024     # 4 exp chunks of 1024 per m-chunk
NQ = N // 512      # 8 AV n-chunks of 512

Exp = mybir.ActivationFunctionType.Exp
Sigmoid = mybir.ActivationFunctionType.Sigmoid
ADD = mybir.AluOpType.add
AX = mybir.AxisListType.X
PSUM = bass.MemorySpace.PSUM


def build():
    nc = bacc.Bacc("TRN2", target_bir_lowering=False, debug=False, num_devices=H)

    # per-core column slice of X: core c gets X[:, 128c:128(c+1)]
    xcol_d = nc.dram_tensor("xcol", [N, 128], F32, kind="ExternalInput")
    wq_d = nc.dram_tensor("wq", [D, E], F32, kind="ExternalInput")
    wk_d = nc.dram_tensor("wk", [D, E], F32, kind="ExternalInput")
    wv_d = nc.dram_tensor("wv", [D, E], F32, kind="ExternalInput")
    bq_d = nc.dram_tensor("bq", [E, 1], F32, kind="ExternalInput")
    bk_d = nc.dram_tensor("bk", [E, 1], F32, kind="ExternalInput")
    bv_d = nc.dram_tensor("bv", [E, 1], F32, kind="ExternalInput")
    out_d = nc.dram_tensor("out", [N, E], F32, kind="ExternalOutput")

    eye_bf_d = nc.inline_tensor(np.eye(128, dtype=ml_dtypes.bfloat16), "eye_bf")
    eye_f_d = nc.inline_tensor(np.eye(128, dtype=np.float32), "eye_f")

    with tile.TileContext(nc) as tc:
        with tc.tile_pool(name="persist", bufs=1) as persist:
            # --- constants / weights (f32 -> bf16 cast done by SWDGE DMA) ---
            eye_bf = persist.tile([128, 128], BF16, tag="eye_bf")
            nc.sync.dma_start(eye_bf[:], eye_bf_d[:])
            eye_f = persist.tile([128, 128], F32, tag="eye_f")
            nc.sync.dma_start(eye_f[:], eye_f_d[:])

            w_sbs = []
            for name, w_d in (("wq", wq_d), ("wk", wk_d), ("wv", wv_d)):
                w_sb = persist.tile([128, DC, E], BF16, tag=name)
                nc.gpsimd.dma_start(
                    w_sb[:], w_d.ap().rearrange("(c p) e -> p c e", p=128)
                )
                w_sbs.append(w_sb)
            b_sbs = []
            for name, b_d in (("bq", bq_d), ("bk", bk_d), ("bv", bv_d)):
                b_sb = persist.tile([E, 1], F32, tag=name)
                nc.sync.dma_start(b_sb[:], b_d[:])
                b_sbs.append(b_sb)

            qT = persist.tile([E, N], BF16, tag="qT")
            kT = persist.tile([E, N], BF16, tag="kT")
            vT = persist.tile([E, N], BF16, tag="vT")
            v_sb = persist.tile([128, MC, E], BF16, tag="v")
            zT = persist.tile([E, N], F32, tag="zT")
            projT = (qT, kT, vT)

            # --- phase 1a: transpose MY d-slice of X, AllGather full XT ---
            with (
                tc.tile_pool(name="xload", bufs=1) as xload,
                tc.tile_pool(name="xtp", bufs=1) as xtp,
                tc.tile_pool(name="dram", bufs=1, space=bass.MemorySpace.DRAM) as dram,
                tc.tile_pool(name="trps", bufs=2, space=PSUM) as trp,
                tc.tile_pool(name="pjps", bufs=3, space=PSUM) as pjp,
            ):
                xt_shard = dram.tile([128, N], BF16, name="xt_shard")
                xt_all = dram.tile([D, N], BF16, addr_space="Shared", name="xt_all")

                # load my 128-col slice of X (f32 -> bf16 cast in DMA)
                xc = xload.tile([128, 32, 128], BF16, tag="xc")
                nc.gpsimd.dma_start(
                    xc[:], xcol_d.ap().rearrange("(t p) d -> p t d", p=128)
                )
                xt_my = xload.tile([128, N], BF16, tag="xt_my")
                for grp in range(8):
                    ps = trp.tile([128, 4, 128], BF16, tag="trps")
                    for j in range(4):
                        nc.tensor.transpose(ps[:, j, :], xc[:, grp * 4 + j, :], eye_bf[:])
                    nc.vector.tensor_copy(
                        xt_my[:, grp * 512 : (grp + 1) * 512], ps[:]
                    )
                nc.sync.dma_start(xt_shard[:], xt_my[:])
                nc.gpsimd.collective_compute(
                    "AllGather",
                    mybir.AluOpType.bypass,
                    replica_groups=[list(range(H))],
                    ins=[xt_shard.opt()],
                    outs=[xt_all.opt()],
                )

                # --- phase 1b: pull gathered XT, projections qT/kT/vT [e, n] ---
                xt = [
                    xtp.tile([128, N], BF16, tag=f"xt{dc}", name=f"xt{dc}")
                    for dc in range(DC)
                ]
                for gg in range(4):
                    gsl = slice(gg * 1024, (gg + 1) * 1024)
                    for dc in range(DC):
                        nc.sync.dma_start(
                            xt[dc][:, gsl],
                            xt_all[dc * 128 : (dc + 1) * 128, gsl],
                        )
                    for g in (2 * gg, 2 * gg + 1):
                        sl = slice(g * 512, (g + 1) * 512)
                        for w_sb, b_sb, dstT in zip(w_sbs, b_sbs, projT):
                            pp = pjp.tile([128, 512], F32, tag="pj")
                            for dc in range(DC):
                                nc.tensor.matmul(
                                    pp[:],
                                    w_sb[:, dc, :],
                                    xt[dc][:, sl],
                                    start=(dc == 0),
                                    stop=(dc == DC - 1),
                                )
                            nc.vector.tensor_scalar_add(dstT[:, sl], pp[:], b_sb[:])

            # --- phase 2: scores^T -> exp (+row-sum) -> AV into zT ---
            with (
                tc.tile_pool(name="expt", bufs=11) as expp,
                tc.tile_pool(name="vsp", bufs=18) as vsp,
                tc.tile_pool(name="dsp", bufs=40) as dsp,
                tc.tile_pool(name="Tps", bufs=3, space=PSUM) as Tp,
                tc.tile_pool(name="avps", bufs=2, space=PSUM) as avp,
            ):
                expts = {}
                recs = {}
                vss = {}

                def emit_v_transposes():
                    for grp in range(8):
                        ps = Tp.tile([128, 4, 128], BF16, tag="T", name=f"vtr{grp}")
                        for j in range(4):
                            mc = grp * 4 + j
                            nc.tensor.transpose(
                                ps[:, j, :], vT[:, mc * 128 : (mc + 1) * 128], eye_bf[:]
                            )
                        nc.vector.tensor_copy(v_sb[:, grp * 4 : grp * 4 + 4, :], ps[:])

                def emit_t_exp_block(b):
                    for j in range(8):
                        mc = b * 8 + j
                        et = expp.tile([128, N], BF16, tag="expt")
                        ds = dsp.tile([128, CH], F32, tag="ds")
                        for ch in range(CH):
                            tp = Tp.tile([128, 1024], F32, tag="T")
                            for h2 in range(2):
                                nc.tensor.matmul(
                                    tp[:, h2 * 512 : (h2 + 1) * 512],
                                    kT[:, mc * 128 : (mc + 1) * 128],
                                    qT[:, ch * 1024 + h2 * 512 : ch * 1024 + (h2 + 1) * 512],
                                    start=True,
                                    stop=True,
                                )
                            nc.scalar.activation(
                                et[:, ch * 1024 : (ch + 1) * 1024],
                                tp[:],
                                Exp,
                                scale=SCALE,
                                accum_out=ds[:, ch : ch + 1],
                            )
                        den = dsp.tile([128, 1], F32, tag="den")
                        nc.vector.tensor_reduce(den[:], ds[:], AX, ADD)
                        rec = dsp.tile([128, 1], F32, tag="rec")
                        nc.vector.reciprocal(rec[:], den[:])
                        expts[mc] = et
                        recs[mc] = rec

                def emit_av_block(b):
                    for j in range(8):
                        mc = b * 8 + j
                        vs = vsp.tile([128, E], BF16, tag="vs")
                        nc.vector.tensor_scalar_mul(vs[:], v_sb[:, mc, :], recs[mc][:])
                        vss[mc] = vs
                    for nq in range(NQ):
                        sl = slice(nq * 512, (nq + 1) * 512)
                        ap = avp.tile([128, 512], F32, tag="av")
                        for j in range(8):
                            mc = b * 8 + j
                            nc.tensor.matmul(
                                ap[:],
                                vss[mc][:],
                                expts[mc][:, sl],
                                start=(j == 0),
                                stop=(j == 7),
                            )
                        if b == 0:
                            nc.vector.tensor_copy(zT[:, sl], ap[:])
                        else:
                            nc.vector.tensor_tensor(zT[:, sl], zT[:, sl], ap[:], ADD)

                emit_t_exp_block(0)
                emit_v_transposes()
                for b in range(1, MB):
                    emit_t_exp_block(b)
                    emit_av_block(b - 1)
                emit_av_block(MB - 1)

            # --- phase 3: sigmoid, transpose back to [n, e], DMA out ---
            with (
                tc.tile_pool(name="outp", bufs=2) as outp,
                tc.tile_pool(name="ops", bufs=2, space=PSUM) as ops,
            ):
                nc.scalar.activation(zT[:], zT[:], Sigmoid)
                out_view = out_d.ap().rearrange("(g j p) e -> g p j e", p=128, j=4)
                for g in range(8):
                    ps = ops.tile([128, 4, 128], F32, tag="ops")
                    for j in range(4):
                        nt = g * 4 + j
                        nc.tensor.transpose(
                            ps[:, j, :], zT[:, nt * 128 : (nt + 1) * 128], eye_f[:]
                        )
                    ot = outp.tile([128, 4, 128], F32, tag="ot")
                    nc.vector.tensor_copy(ot[:], ps[:])
                    nc.sync.dma_start(out_view[g], ot[:])

    nc.compile()
    return nc


_NC = None


def _get_nc():
    global _NC
    if _NC is None:
        _NC = build()
    return _NC


def _make_in_maps(inputs):
    X = np.ascontiguousarray(np.asarray(inputs["X"], dtype=np.float32))
    Wq = np.asarray(inputs["Wq"], dtype=np.float32)
    Wk = np.asarray(inputs["Wk"], dtype=np.float32)
    Wv = np.asarray(inputs["Wv"], dtype=np.float32)
    bq = np.asarray(inputs["bq"], dtype=np.float32)
    bk = np.asarray(inputs["bk"], dtype=np.float32)
    bv = np.asarray(inputs["bv"], dtype=np.float32)
    in_maps = []
    for h in range(H):
        in_maps.append(
            {
                "xcol": np.ascontiguousarray(X[:, h * 128 : (h + 1) * 128]),
                "wq": np.ascontiguousarray(Wq[h]),
                "wk": np.ascontiguousarray(Wk[h]),
                "wv": np.ascontiguousarray(Wv[h]),
                "bq": np.ascontiguousarray(bq[h].reshape(E, 1)),
                "bk": np.ascontiguousarray(bk[h].reshape(E, 1)),
                "bv": np.ascontiguousarray(bv[h].reshape(E, 1)),
            }
        )
    return in_maps


def run(inputs, trace=False, tmpdir=None):
    nc = _get_nc()
    res = run_bass_kernel_spmd(
        nc, _make_in_maps(inputs), list(range(H)), trace=trace, tmpdir=tmpdir
    )
    out = np.concatenate([res.results[h]["out"] for h in range(H)], axis=1)
    return out.astype(np.float32), res


def kernel(**inputs) -> np.ndarray:
    out, _ = run(inputs)
    return out


# revision 14
# speedup vs baseline: 1.3478x; 1.3478x over previous
"""Head-parallel multi-head attention kernel for 8 TRN2 NeuronCores.

Problem: X[4096,1024] @ per-head Wq/Wk/Wv[1024,128] (+bias) -> per-head
scores S = q k^T * SCALE, softmax over the QUERY axis (axis n), z = attn @ v,
concat heads, sigmoid.  H=8 heads -> 1 head per core, zero collectives.

Per-core algorithm (head h):
  - Work in transposed space: T = S^T laid out [m(part), n(free)] so the
    softmax reduction is a free-axis row-sum.
  - softmax normalization folded into v:  z^T[e,n] = sum_m exp(T[m,n]) *
    (v[m,e]/denom[m]); denom comes free from the ACT accum_out of the exp.
  - zT accumulated in PSUM per 8-chunk m-block, flushed to SBUF by DVE.
  - sigmoid on zT, PE-transpose back to [n,e], DMA out.
Host: shard weights per head, replicate X, concat core outputs on axis=1.
"""

import numpy as np
import ml_dtypes

from concourse import bacc, bass, tile, mybir
from concourse.bass_utils import run_bass_kernel_spmd

N, D, E = 4096, 1024, 128
H = 8
SCALE = 0.08838834764831845
BF16 = mybir.dt.bfloat16
F32 = mybir.dt.float32

DC = D // 128      # 8 d-chunks of 128 (contraction tiles)
NG = N // 512      # 8 column groups of 512
MC = N // 128      # 32 m-chunks of 128
MB = MC // 8       # 4 m-blocks of 8 chunks
CH = N // 1024     # 4 exp chunks of 1024 per m-chunk
NQ = N // 512      # 8 AV n-chunks of 512

Exp = mybir.ActivationFunctionType.Exp
Sigmoid = mybir.ActivationFunctionType.Sigmoid
ADD = mybir.AluOpType.add
AX = mybir.AxisListType.X
PSUM = bass.MemorySpace.PSUM


def build():
    nc = bacc.Bacc("TRN2", target_bir_lowering=False, debug=False, num_devices=H)

    x_d = nc.dram_tensor("x", [N, D], F32, kind="ExternalInput")
    wq_d = nc.dram_tensor("wq", [D, E], F32, kind="ExternalInput")
    wk_d = nc.dram_tensor("wk", [D, E], F32, kind="ExternalInput")
    wv_d = nc.dram_tensor("wv", [D, E], F32, kind="ExternalInput")
    bq_d = nc.dram_tensor("bq", [E, 1], F32, kind="ExternalInput")
    bk_d = nc.dram_tensor("bk", [E, 1], F32, kind="ExternalInput")
    bv_d = nc.dram_tensor("bv", [E, 1], F32, kind="ExternalInput")
    out_d = nc.dram_tensor("out", [N, E], F32, kind="ExternalOutput")

    eye_bf_d = nc.inline_tensor(np.eye(128, dtype=ml_dtypes.bfloat16), "eye_bf")
    eye_f_d = nc.inline_tensor(np.eye(128, dtype=np.float32), "eye_f")

    with tile.TileContext(nc) as tc:
        with tc.tile_pool(name="persist", bufs=1) as persist:
            # --- constants / weights (f32 -> bf16 cast done by SWDGE DMA) ---
            eye_bf = persist.tile([128, 128], BF16, tag="eye_bf")
            nc.sync.dma_start(eye_bf[:], eye_bf_d[:])
            eye_f = persist.tile([128, 128], F32, tag="eye_f")
            nc.sync.dma_start(eye_f[:], eye_f_d[:])

            w_sbs = []
            for name, w_d in (("wq", wq_d), ("wk", wk_d), ("wv", wv_d)):
                w_sb = persist.tile([128, DC, E], BF16, tag=name)
                nc.gpsimd.dma_start(
                    w_sb[:], w_d.ap().rearrange("(c p) e -> p c e", p=128)
                )
                w_sbs.append(w_sb)
            b_sbs = []
            for name, b_d in (("bq", bq_d), ("bk", bk_d), ("bv", bv_d)):
                b_sb = persist.tile([E, 1], F32, tag=name)
                nc.sync.dma_start(b_sb[:], b_d[:])
                b_sbs.append(b_sb)

            qT = persist.tile([E, N], BF16, tag="qT")
            kT = persist.tile([E, N], BF16, tag="kT")
            vT = persist.tile([E, N], BF16, tag="vT")
            v_sb = persist.tile([128, MC, E], BF16, tag="v")
            zT = persist.tile([E, N], F32, tag="zT")
            projT = (qT, kT, vT)

            # --- phase 1: X -> XT (bf16) via regular-matmul transposes
            # (identity rhs; runs ~3x faster than transpose-mode AND engages
            # the HAM clock so the PE warms to 2.4 GHz), then projections ---
            x_view = x_d.ap().rearrange("(g t p) d -> g p t d", p=128, t=4)
            with (
                tc.tile_pool(name="xload", bufs=3) as xload,
                tc.tile_pool(name="xtp", bufs=1) as xtp,
                tc.tile_pool(name="trps", bufs=2, space=PSUM) as trp,
                tc.tile_pool(name="pjps", bufs=3, space=PSUM) as pjp,
            ):
                xt = [
                    xtp.tile([128, N], BF16, tag=f"xt{dc}", name=f"xt{dc}")
                    for dc in range(DC)
                ]
                for g in range(NG):
                    xb = xload.tile([128, 4, D], BF16, tag="xb")
                    nc.gpsimd.dma_start(xb[:], x_view[g])
                    sl = slice(g * 512, (g + 1) * 512)
                    for dc in range(DC):
                        ps = trp.tile([128, 4, 128], F32, tag="trps")
                        for j in range(4):
                            nc.tensor.matmul(
                                ps[:, j, :],
                                xb[:, j, dc * 128 : (dc + 1) * 128],
                                eye_bf[:],
                                start=True,
                                stop=True,
                            )
                        nc.vector.tensor_copy(xt[dc][:, sl], ps[:])
                    for w_sb, b_sb, dstT in zip(w_sbs, b_sbs, projT):
                        pp = pjp.tile([128, 512], F32, tag="pj")
                        for dc in range(DC):
                            nc.tensor.matmul(
                                pp[:],
                                w_sb[:, dc, :],
                                xt[dc][:, sl],
                                start=(dc == 0),
                                stop=(dc == DC - 1),
                            )
                        nc.vector.tensor_scalar_add(dstT[:, sl], pp[:], b_sb[:])

            # --- phase 2: scores^T -> exp (+row-sum) -> AV into zT ---
            with (
                tc.tile_pool(name="expt", bufs=11) as expp,
                tc.tile_pool(name="vsp", bufs=18) as vsp,
                tc.tile_pool(name="dsp", bufs=40) as dsp,
                tc.tile_pool(name="Tps", bufs=3, space=PSUM) as Tp,
                tc.tile_pool(name="avps", bufs=2, space=PSUM) as avp,
            ):
                expts = {}
                recs = {}
                vss = {}

                def emit_v_transposes():
                    for grp in range(8):
                        ps = Tp.tile([128, 4, 128], F32, tag="T", name=f"vtr{grp}")
                        for j in range(4):
                            mc = grp * 4 + j
                            nc.tensor.matmul(
                                ps[:, j, :],
                                vT[:, mc * 128 : (mc + 1) * 128],
                                eye_bf[:],
                                start=True,
                                stop=True,
                            )
                        nc.vector.tensor_copy(v_sb[:, grp * 4 : grp * 4 + 4, :], ps[:])

                def emit_t_exp_block(b):
                    for j in range(8):
                        mc = b * 8 + j
                        et = expp.tile([128, N], BF16, tag="expt")
                        ds = dsp.tile([128, CH], F32, tag="ds")
                        for ch in range(CH):
                            tp = Tp.tile([128, 1024], F32, tag="T")
                            for h2 in range(2):
                                nc.tensor.matmul(
                                    tp[:, h2 * 512 : (h2 + 1) * 512],
                                    kT[:, mc * 128 : (mc + 1) * 128],
                                    qT[:, ch * 1024 + h2 * 512 : ch * 1024 + (h2 + 1) * 512],
                                    start=True,
                                    stop=True,
                                )
                            nc.scalar.activation(
                                et[:, ch * 1024 : (ch + 1) * 1024],
                                tp[:],
                                Exp,
                                scale=SCALE,
                                accum_out=ds[:, ch : ch + 1],
                            )
                        den = dsp.tile([128, 1], F32, tag="den")
                        nc.vector.tensor_reduce(den[:], ds[:], AX, ADD)
                        rec = dsp.tile([128, 1], F32, tag="rec")
                        nc.vector.reciprocal(rec[:], den[:])
                        expts[mc] = et
                        recs[mc] = rec

                def emit_av_block(b):
                    for j in range(8):
                        mc = b * 8 + j
                        vs = vsp.tile([128, E], BF16, tag="vs")
                        nc.vector.tensor_scalar_mul(vs[:], v_sb[:, mc, :], recs[mc][:])
                        vss[mc] = vs
                    for nq in range(NQ):
                        sl = slice(nq * 512, (nq + 1) * 512)
                        ap = avp.tile([128, 512], F32, tag="av")
                        for j in range(8):
                            mc = b * 8 + j
                            nc.tensor.matmul(
                                ap[:],
                                vss[mc][:],
                                expts[mc][:, sl],
                                start=(j == 0),
                                stop=(j == 7),
                            )
                        if b == 0:
                            nc.vector.tensor_copy(zT[:, sl], ap[:])
                        else:
                            nc.vector.tensor_tensor(zT[:, sl], zT[:, sl], ap[:], ADD)

                emit_t_exp_block(0)
                emit_v_transposes()
                for b in range(1, MB):
                    emit_t_exp_block(b)
                    emit_av_block(b - 1)
                emit_av_block(MB - 1)

            # --- phase 3: sigmoid (bf16), transpose back to [n, e], DMA out ---
            with (
                tc.tile_pool(name="outp", bufs=2) as outp,
                tc.tile_pool(name="ops", bufs=2, space=PSUM) as ops,
            ):
                zs = persist.tile([E, N], BF16, tag="zs")
                nc.scalar.activation(zs[:], zT[:], Sigmoid)
                out_view = out_d.ap().rearrange("(g j p) e -> g p j e", p=128, j=4)
                for g in range(8):
                    ps = ops.tile([128, 4, 128], F32, tag="ops")
                    for j in range(4):
                        nt = g * 4 + j
                        nc.tensor.matmul(
                            ps[:, j, :],
                            zs[:, nt * 128 : (nt + 1) * 128],
                            eye_bf[:],
                            start=True,
                            stop=True,
                        )
                    ot = outp.tile([128, 4, 128], F32, tag="ot")
                    nc.vector.tensor_copy(ot[:], ps[:])
                    nc.sync.dma_start(out_view[g], ot[:])

    nc.compile()
    return nc


_NC = None


def _get_nc():
    global _NC
    if _NC is None:
        _NC = build()
    return _NC


def _make_in_maps(inputs):
    X = np.ascontiguousarray(np.asarray(inputs["X"], dtype=np.float32))
    Wq = np.asarray(inputs["Wq"], dtype=np.float32)
    Wk = np.asarray(inputs["Wk"], dtype=np.float32)
    Wv = np.asarray(inputs["Wv"], dtype=np.float32)
    bq = np.asarray(inputs["bq"], dtype=np.float32)
    bk = np.asarray(inputs["bk"], dtype=np.float32)
    bv = np.asarray(inputs["bv"], dtype=np.float32)
    in_maps = []
    for h in range(H):
        in_maps.append(
            {
                "x": X,
                "wq": np.ascontiguousarray(Wq[h]),
                "wk": np.ascontiguousarray(Wk[h]),
                "wv": np.ascontiguousarray(Wv[h]),
                "bq": np.ascontiguousarray(bq[h].reshape(E, 1)),
                "bk": np.ascontiguousarray(bk[h].reshape(E, 1)),
                "bv": np.ascontiguousarray(bv[h].reshape(E, 1)),
            }
        )
    return in_maps


def run(inputs, trace=False, tmpdir=None):
    nc = _get_nc()
    res = run_bass_kernel_spmd(
        nc, _make_in_maps(inputs), list(range(H)), trace=trace, tmpdir=tmpdir
    )
    out = np.concatenate([res.results[h]["out"] for h in range(H)], axis=1)
    return out.astype(np.float32), res


def kernel(**inputs) -> np.ndarray:
    out, _ = run(inputs)
    return out
